# revision 22
# baseline (speedup 1.0000x reference)
"""Trainium2 Bass kernel for nn_AoEBlock (moe_routing).

Sharding: data-parallel over batch B=8 across the 8 NeuronCores (core b
handles image b = 1024 tokens). Weights are replicated per core (bf16 for
the heavy matmuls, f32 where routing precision matters). No collectives;
the aux-loss needs only per-core partial sums ([8,2] floats per core)
which are combined on gather.

Math notes:
 - shared expert + AoE + residual all accumulate into the same PSUM banks:
     out = x + w2t.T@gelu(w1t.T@x + b1) + b2 + wup.T@(gelu(wdt.T@x) * G)
   where G[f, t] = gate(e(f), t) is the dense per-(expert,token) gate
   (0 when the expert is not in the token's top-2), broadcast from an
   [8, T] gate matrix via a tiny selection matmul.
 - router logits are computed exactly in f32 as x.T @ rp where
   rp[e, c] = sum_d router_w[d] * w_down[e*192+d, c] (computed on device
   from the f32 w_down, using a zero-padded per-expert router vector so
   full 128-row k-tiles accumulate cleanly).
 - top-2 renormalized softmax weights: g1 = 1/(1+exp(l2-l1)), g2 = 1-g1
   (exact identity; uses the Exp table already loaded for the softmax).

Schedule: quarter-0 shared-expert work is emitted first so the PE gets
dense matmul work as soon as the first weight chunks land (keeps the HAM
clock warm); router projection + all routing runs next (its DMAs overlap
the shared-expert matmuls); MoE + remaining quarters follow.
"""

import sys
import numpy as np

for _p in ("/opt/trn_rl_repo", "/root/.axon_site"):
    if _p not in sys.path:
        sys.path.insert(0, _p)

import ml_dtypes

import concourse.bass as bass
import concourse.bacc as bacc
import concourse.mybir as mybir
import concourse.tile as tile
from concourse.bass_utils import run_bass_kernel_spmd

F32 = mybir.dt.float32
BF16 = mybir.dt.bfloat16
AF = mybir.ActivationFunctionType
ALU = mybir.AluOpType
AX = mybir.AxisListType

NCORES = 8
C = 768            # model dim
KC = C // 128      # 6 c k-tiles
HID = 3072         # shared expert hidden
MH = HID // 128    # 24 hidden tiles
E = 8              # experts
DL = 192           # d_low
F = E * DL         # 1536 flattened expert-feature dim
JF = F // 128      # 12 feats tiles
T = 1024           # tokens per core
NQ = 4             # token chunks
TQ = T // NQ       # 256 tokens per chunk
BIGNEG = -1.0e30


def build_nc():
    nc = bacc.Bacc("TRN2", target_bir_lowering=False, debug=False,
                   num_devices=NCORES)

    # ---- DRAM parameters (per-core shards) ----
    x32_h = nc.dram_tensor("x32", [C, T], F32, kind="ExternalInput")
    xb_h = nc.dram_tensor("xb", [C, T], BF16, kind="ExternalInput")
    w1t_h = nc.dram_tensor("w1t", [C, HID], BF16, kind="ExternalInput")
    w2t_h = nc.dram_tensor("w2t", [HID, C], BF16, kind="ExternalInput")
    wdt_h = nc.dram_tensor("wdt", [C, F], BF16, kind="ExternalInput")
    wup_h = nc.dram_tensor("wup", [F, C], BF16, kind="ExternalInput")
    wd32_h = nc.dram_tensor("wd32", [F, C], F32, kind="ExternalInput")
    rbig8_h = nc.dram_tensor("rbig8", [128, JF * E], F32, kind="ExternalInput")
    selm_h = nc.dram_tensor("selm", [E, F], BF16, kind="ExternalInput")
    b1m_h = nc.dram_tensor("b1m", [128, MH], F32, kind="ExternalInput")
    b2m_h = nc.dram_tensor("b2m", [128, KC], F32, kind="ExternalInput")
    ident_h = nc.dram_tensor("ident", [128, 128], F32, kind="ExternalInput")
    out_h = nc.dram_tensor("out", [C, T], F32, kind="ExternalOutput")
    aux_h = nc.dram_tensor("aux", [E, 2], F32, kind="ExternalOutput")

    with tile.TileContext(nc) as tc:
        with (
            tc.tile_pool(name="const", bufs=1) as constp,
            tc.tile_pool(name="big", bufs=1) as bigp,
            tc.tile_pool(name="gh", bufs=3) as ghp,
            tc.tile_pool(name="gf", bufs=3) as gfp,
            tc.tile_pool(name="ga", bufs=3) as gap,
            tc.tile_pool(name="ev", bufs=3) as evp,
            tc.tile_pool(name="rt", bufs=2) as rtp,
            tc.tile_pool(name="po", bufs=3, space=bass.MemorySpace.PSUM) as pop,
            tc.tile_pool(name="ph", bufs=3, space=bass.MemorySpace.PSUM) as php,
            tc.tile_pool(name="ps", bufs=2, space=bass.MemorySpace.PSUM) as psp,
        ):
            # ---- persistent SBUF tensors ----
            x_sb = constp.tile([128, KC * T], F32, tag="x32")
            xb_sb = constp.tile([128, KC * T], BF16, tag="xb")
            w1t_sb = constp.tile([128, KC * HID], BF16, tag="w1t")
            w2t_sb = constp.tile([128, MH * C], BF16, tag="w2t")
            wdt_sb = constp.tile([128, KC * F], BF16, tag="wdt")
            wup_sb = constp.tile([128, JF * C], BF16, tag="wup")
            # wd32 dies after the router projection; the shared-expert
            # partials are born after it — share one big slot.
            wd32_sb = bigp.tile([128, JF * C], F32, tag="big", name="wd32")
            rbig8_sb = constp.tile([128, JF * E], F32, tag="rbig8")
            selm_sb = constp.tile([E, F], BF16, tag="selm")
            b1_sb = constp.tile([128, MH], F32, tag="b1m")
            b2_sb = constp.tile([128, KC], F32, tag="b2m")
            ident_sb = constp.tile([128, 128], F32, tag="ident")
            rpt_sb = constp.tile([128, KC * E], F32, tag="rpt")
            gate_sb = constp.tile([E, T], BF16, tag="gate")
            ones_sb = constp.tile([128, 1], F32, tag="ones")
            aux_sb = constp.tile([E, 2], F32, tag="auxs")
            shared_sb = bigp.tile([128, KC * T], BF16, tag="big",
                                  name="shared")

            # ---- DMA loads, ordered by first consumption ----
            # Merged 3D-access-pattern transfers keep the per-dma_start issue
            # cost (~0.6us on the sync sequencer) off the critical path.
            xb_dst = xb_sb[:].rearrange("p (k t) -> p k t", k=KC)
            xb_src = xb_h[:].rearrange("(k p) t -> p k t", p=128)
            x32_dst = x_sb[:].rearrange("p (k t) -> p k t", k=KC)
            x32_src = x32_h[:].rearrange("(k p) t -> p k t", p=128)
            w1t_dst = w1t_sb[:].rearrange("p (k w) -> p k w", k=KC)
            w1t_src = w1t_h[:].rearrange("(k p) w -> p k w", p=128)
            w2t_dst = w2t_sb[:].rearrange("p (m c) -> p m c", m=MH)
            w2t_src = w2t_h[:].rearrange("(m p) c -> p m c", p=128)
            wd32_dst = wd32_sb[:].rearrange("p (j c) -> p j c", j=JF)
            wd32_src = wd32_h[:].rearrange("(j p) c -> p j c", p=128)
            wdt_dst = wdt_sb[:].rearrange("p (k f) -> p k f", k=KC)
            wdt_src = wdt_h[:].rearrange("(k p) f -> p k f", p=128)
            wup_dst = wup_sb[:].rearrange("p (j c) -> p j c", j=JF)
            wup_src = wup_h[:].rearrange("(j p) c -> p j c", p=128)

            nc.sync.dma_start(b1_sb[:], b1m_h[:])
            nc.sync.dma_start(xb_dst[:, :, 0:TQ], xb_src[:, :, 0:TQ])
            # w1t / w2t interleaved in hidden-tile (consumption) order
            for g in range(6):
                w0, w1 = g * (HID // 6), (g + 1) * (HID // 6)
                nc.sync.dma_start(w1t_dst[:, :, w0:w1], w1t_src[:, :, w0:w1])
                m0, m1 = g * (MH // 6), (g + 1) * (MH // 6)
                nc.sync.dma_start(w2t_dst[:, m0:m1, :], w2t_src[:, m0:m1, :])
                if g == 0:  # rest of xb right behind the first weight chunk
                    nc.sync.dma_start(xb_dst[:, :, TQ:T], xb_src[:, :, TQ:T])
            nc.vector.memset(ones_sb[:], 1.0)
            nc.vector.memset(aux_sb[:], 0.0)
            nc.sync.dma_start(b2_sb[:], b2m_h[:])
            nc.sync.dma_start(rbig8_sb[:], rbig8_h[:])
            nc.sync.dma_start(selm_sb[:], selm_h[:])
            nc.sync.dma_start(ident_sb[:], ident_h[:])
            # router path + x32 (consumed mid-kernel)
            for g in range(4):
                j0, j1 = g * (JF // 4), (g + 1) * (JF // 4)
                nc.sync.dma_start(wd32_dst[:, j0:j1, :], wd32_src[:, j0:j1, :])
            nc.sync.dma_start(x32_dst[:], x32_src[:])
            # MoE weights (consumed in the back half)
            for g in range(2):
                k0, k1 = g * (KC // 2), (g + 1) * (KC // 2)
                nc.sync.dma_start(wdt_dst[:, k0:k1, :], wdt_src[:, k0:k1, :])
            for g in range(2):
                j0, j1 = g * (JF // 2), (g + 1) * (JF // 2)
                nc.sync.dma_start(wup_dst[:, j0:j1, :], wup_src[:, j0:j1, :])

            # ---- helpers ----
            # Both heavy loops are software-pipelined by one step: the
            # consumer matmuls (mm2 / expert) for step i are emitted after
            # the producer matmuls of step i+1, so the ACT/DVE latency of
            # gelu/gating is covered by PE work and the PE never waits.
            def shared_expert_phase(q, po, out_slice, pending, m_lo=0,
                                    m_hi=MH, flush=False):
                def mm2(m, gh):
                    for ct in range(KC):
                        nc.tensor.matmul(
                            out_slice(ct),
                            w2t_sb[:, m * C + ct * 128: m * C + ct * 128 + 128],
                            gh[:],
                            start=(m == 0 and ct % 2 == 0),
                            stop=(m == MH - 1 and ct % 2 == 1),
                        )
                for m in range(m_lo, m_hi):
                    h_ps = php.tile([128, TQ], F32, tag="hf", name=f"h{q}_{m}")
                    for k in range(KC):
                        nc.tensor.matmul(
                            h_ps[:],
                            w1t_sb[:, k * HID + m * 128: k * HID + m * 128 + 128],
                            xb_sb[:, k * T + q * TQ: k * T + q * TQ + TQ],
                            start=(k == 0), stop=(k == KC - 1),
                        )
                    gh = ghp.tile([128, TQ], BF16, tag="gh", name=f"gh{q}_{m}")
                    nc.scalar.activation(gh[:], h_ps[:], AF.Gelu,
                                         bias=b1_sb[:, m:m + 1])
                    if pending:
                        mm2(*pending.pop())
                    pending.append((m, gh))
                if flush:
                    while pending:
                        mm2(*pending.pop())

            def moe_phase(q, po, out_slice):
                pending = []

                def expert(j, ga):
                    for ct in range(KC):
                        nc.tensor.matmul(
                            out_slice(ct),
                            wup_sb[:, j * C + ct * 128: j * C + ct * 128 + 128],
                            ga[:],
                            start=(j == 0 and ct % 2 == 0),
                            stop=(j == JF - 1 and ct % 2 == 1),
                        )
                for j in range(JF):
                    f_ps = php.tile([128, TQ], F32, tag="hf", name=f"f{q}_{j}")
                    for k in range(KC):
                        nc.tensor.matmul(
                            f_ps[:],
                            wdt_sb[:, k * F + j * 128: k * F + j * 128 + 128],
                            xb_sb[:, k * T + q * TQ: k * T + q * TQ + TQ],
                            start=(k == 0), stop=(k == KC - 1),
                        )
                    gf = gfp.tile([128, TQ], BF16, tag="gf", name=f"gf{q}_{j}")
                    nc.scalar.activation(gf[:], f_ps[:], AF.Gelu)
                    g_ps = php.tile([128, TQ], F32, tag="hf", name=f"g{q}_{j}")
                    nc.tensor.matmul(g_ps[:],
                                     selm_sb[:, j * 128:(j + 1) * 128],
                                     gate_sb[:, q * TQ:(q + 1) * TQ],
                                     start=True, stop=True)
                    ga = gap.tile([128, TQ], BF16, tag="ga", name=f"ga{q}_{j}")
                    nc.vector.tensor_mul(ga[:], gf[:], g_ps[:])
                    if pending:
                        expert(*pending.pop())
                    pending.append((j, ga))
                while pending:
                    expert(*pending.pop())

            def evict_shared(q, out_slice):
                # park the shared-expert accumulation in SBUF (bf16), freeing
                # the po banks for the next quarter
                for ct in range(KC):
                    nc.scalar.copy(
                        shared_sb[:, ct * T + q * TQ: ct * T + q * TQ + TQ],
                        out_slice(ct))

            def evict_phase(q, po, out_slice):
                # out = moe_psum + x + shared + b2
                for ct in range(KC):
                    t1 = evp.tile([128, TQ], F32, tag="t1", name=f"t1{q}_{ct}")
                    nc.vector.tensor_add(
                        t1[:], out_slice(ct),
                        x_sb[:, ct * T + q * TQ: ct * T + q * TQ + TQ])
                    ev = evp.tile([128, TQ], F32, tag="ev", name=f"ev{q}_{ct}")
                    nc.vector.scalar_tensor_tensor(
                        ev[:], t1[:], b2_sb[:, ct:ct + 1],
                        shared_sb[:, ct * T + q * TQ: ct * T + q * TQ + TQ],
                        ALU.add, ALU.add,
                    )
                    nc.sync.dma_start(
                        out_h[ct * 128:(ct + 1) * 128, q * TQ:(q + 1) * TQ],
                        ev[:])

            def make_po(q):
                po = [pop.tile([128, 2 * TQ], F32, tag="po", name=f"po{q}_{i}")
                      for i in range(3)]

                def out_slice(ct):
                    return po[ct // 2][:, (ct % 2) * TQ:(ct % 2) * TQ + TQ]
                return po, out_slice

            def router_projection():
                # rp[c, ct*8+e] over one psum accumulation region; rbig8 is
                # zero off-expert, so full 128-row k-tiles accumulate the
                # per-expert d-contraction exactly.
                rp_ps = psp.tile([128, KC * E], F32, tag="small", name="rp_ps")
                for ct in range(KC):
                    for kt in range(JF):
                        nc.tensor.matmul(
                            rp_ps[:, ct * E:(ct + 1) * E],
                            wd32_sb[:, kt * C + ct * 128: kt * C + ct * 128 + 128],
                            rbig8_sb[:, kt * E:(kt + 1) * E],
                            start=(ct == 0 and kt == 0),
                            stop=(ct == KC - 1 and kt == JF - 1),
                        )
                nc.vector.tensor_copy(rpt_sb[:], rp_ps[:])

            Ls = {}

            def logits_stage(tt):
                tok0 = tt * 128
                l_ps = psp.tile([128, E], F32, tag="small", name=f"l{tt}")
                for k in range(KC):
                    nc.tensor.matmul(
                        l_ps[:],
                        x_sb[:, k * T + tok0: k * T + tok0 + 128],
                        rpt_sb[:, k * E:(k + 1) * E],
                        start=(k == 0), stop=(k == KC - 1),
                    )
                L = rtp.tile([128, E], F32, tag="L", name=f"L{tt}", bufs=8)
                nc.vector.tensor_copy(L[:], l_ps[:])
                Ls[tt] = L

            routed = {}

            def route_stage(tt):
                # DVE/ACT only: from logits to token-major gates + aux inputs
                L = Ls[tt]
                m1 = rtp.tile([128, 1], F32, tag="m1", name=f"m1_{tt}")
                nm1 = rtp.tile([128, 1], F32, tag="nm1", name=f"nm1_{tt}")
                nc.vector.tensor_reduce(m1[:], L[:], AX.X, ALU.max)
                nc.vector.tensor_reduce(nm1[:], L[:], AX.X, ALU.max,
                                        negate=True)
                s = rtp.tile([128, E], F32, tag="s", name=f"s{tt}")
                nc.scalar.activation(s[:], L[:], AF.Exp, bias=nm1[:, 0:1])
                Z = rtp.tile([128, 1], F32, tag="Z", name=f"Z{tt}")
                nc.vector.tensor_reduce(Z[:], s[:], AX.X, ALU.add)
                rZ = rtp.tile([128, 1], F32, tag="rZ", name=f"rZ{tt}")
                nc.vector.reciprocal(rZ[:], Z[:])
                probs = rtp.tile([128, E], F32, tag="probs", name=f"pr{tt}", bufs=8)
                nc.vector.tensor_scalar_mul(probs[:], s[:], rZ[:, 0:1])
                mask1 = rtp.tile([128, E], F32, tag="mask1", name=f"mk1{tt}")
                nc.vector.tensor_scalar(mask1[:], L[:], m1[:, 0:1], None,
                                        ALU.is_equal)
                L2 = rtp.tile([128, E], F32, tag="L2", name=f"L2_{tt}")
                nc.vector.scalar_tensor_tensor(L2[:], mask1[:], BIGNEG,
                                               L[:], ALU.mult, ALU.add)
                m2 = rtp.tile([128, 1], F32, tag="m2", name=f"m2_{tt}")
                nc.vector.tensor_reduce(m2[:], L2[:], AX.X, ALU.max)
                mask2 = rtp.tile([128, E], F32, tag="mask2", name=f"mk2{tt}")
                nc.vector.tensor_scalar(mask2[:], L2[:], m2[:, 0:1], None,
                                        ALU.is_equal)
                # g1 = 1/(1+exp(l2-l1)); g2 = 1-g1  (same identity as
                # renormalized top-2 softmax; reuses the Exp table)
                em = rtp.tile([128, 1], F32, tag="em", name=f"em{tt}")
                nc.scalar.activation(em[:], m2[:], AF.Exp, bias=nm1[:, 0:1])
                den = rtp.tile([128, 1], F32, tag="den", name=f"den{tt}")
                nc.vector.tensor_scalar_add(den[:], em[:], 1.0)
                g1 = rtp.tile([128, 1], F32, tag="g1", name=f"g1_{tt}")
                nc.vector.reciprocal(g1[:], den[:])
                g2 = rtp.tile([128, 1], F32, tag="g2", name=f"g2_{tt}")
                nc.vector.tensor_scalar(g2[:], g1[:], -1.0, 1.0,
                                        ALU.mult, ALU.add)
                tmp2 = rtp.tile([128, E], F32, tag="tmp2", name=f"t2_{tt}")
                nc.vector.tensor_scalar_mul(tmp2[:], mask2[:], g2[:, 0:1])
                gtok = rtp.tile([128, E], F32, tag="gtok", name=f"gt{tt}", bufs=8)
                nc.vector.scalar_tensor_tensor(gtok[:], mask1[:],
                                               g1[:, 0:1], tmp2[:],
                                               ALU.mult, ALU.add)
                oh = rtp.tile([128, E], F32, tag="oh", name=f"oh{tt}", bufs=8)
                nc.vector.tensor_add(oh[:], mask1[:], mask2[:])
                routed[tt] = (gtok, probs, oh)

            def finish_stage(tt):
                # PE transpose of gates into gate_sb + aux partial sums
                gtok, probs, oh = routed[tt]
                tr_ps = psp.tile([E, 128], F32, tag="small", name=f"tr{tt}")
                nc.tensor.transpose(tr_ps[:], gtok[:], ident_sb[:])
                nc.vector.tensor_copy(gate_sb[:, tt * 128:(tt + 1) * 128],
                                      tr_ps[:])
                aux_ps = psp.tile([E, 2], F32, tag="small", name=f"ax{tt}")
                nc.tensor.matmul(aux_ps[:, 0:1], probs[:], ones_sb[:],
                                 start=True, stop=True)
                nc.tensor.matmul(aux_ps[:, 1:2], oh[:], ones_sb[:],
                                 start=True, stop=True)
                nc.vector.tensor_add(aux_sb[:], aux_sb[:], aux_ps[:])

            # ---- emission schedule ----
            # Quarter-0 shared expert gives the PE dense work immediately;
            # router-projection / logits / routing slot between its segments
            # so their DMA + DVE latencies hide under shared-expert matmuls.
            # Routing chains for token tiles 2-7 (used by quarters 1-3) are
            # emitted after quarter-1's shared-expert loop starts so the DVE
            # never delays quarter-0's gate multiplies.
            # Phase 1: shared expert for ALL quarters back-to-back — pure
            # dense PE work paced only by the w1t/w2t DMA stream. Each
            # quarter's accumulation parks in shared_sb, freeing the po
            # banks. The router (projection, logits, chains, gate builds)
            # slots into the stream where its inputs have surely arrived,
            # long before the MoE phase needs the gates.
            for q in range(NQ):
                po, osl = make_po(q)
                pend = []
                if q == 0:
                    shared_expert_phase(q, po, osl, pend, 0, 12)
                    router_projection()
                    shared_expert_phase(q, po, osl, pend, 12, 24, flush=True)
                elif q == 2:
                    shared_expert_phase(q, po, osl, pend, 0, 8)
                    for tt in range(T // 128):
                        logits_stage(tt)
                    shared_expert_phase(q, po, osl, pend, 8, 24, flush=True)
                    for tt in range(T // 128):
                        route_stage(tt)
                else:
                    shared_expert_phase(q, po, osl, pend, 0, 24, flush=True)
                evict_shared(q, osl)
                if q == 3:
                    for tt in range(T // 128):
                        finish_stage(tt)

            # Phase 2: MoE for all quarters (weights + gates all resident)
            for q in range(NQ):
                po, osl = make_po(q)
                moe_phase(q, po, osl)
                evict_phase(q, po, osl)

            nc.sync.dma_start(aux_h[:], aux_sb[:])

    nc.compile()
    return nc


_CACHE = {}


def _get_nc():
    if "nc" not in _CACHE:
        _CACHE["nc"] = build_nc()
    return _CACHE["nc"]


def _prep_inputs(x, w1, b1, w2, b2, w_down, router_w, w_up):
    bf16 = ml_dtypes.bfloat16
    f32 = np.float32
    x = np.asarray(x, f32)
    # rbig8[p, kt*8+e] = router_w[(kt*128+p) % 192] if (kt*128+p)//192 == e
    flat = np.arange(128)[:, None] + 128 * np.arange(JF)[None, :]  # [128, JF]
    rvals = np.asarray(router_w, f32)[0][flat % DL]                # [128, JF]
    emask = (flat // DL)[:, :, None] == np.arange(E)[None, None, :]
    rbig8 = (rvals[:, :, None] * emask).reshape(128, JF * E).astype(f32)
    shared = {
        "w1t": np.ascontiguousarray(np.asarray(w1, f32).T.astype(bf16)),
        "w2t": np.ascontiguousarray(np.asarray(w2, f32).T.astype(bf16)),
        "wdt": np.ascontiguousarray(np.asarray(w_down, f32).T.astype(bf16)),
        "wup": np.ascontiguousarray(
            np.asarray(w_up, f32).reshape(F, C).astype(bf16)),
        "wd32": np.ascontiguousarray(np.asarray(w_down, f32)),
        "rbig8": np.ascontiguousarray(rbig8),
        "selm": np.ascontiguousarray(
            (np.arange(F)[None, :] // DL == np.arange(E)[:, None])
            .astype(bf16)),
        "b1m": np.ascontiguousarray(
            np.asarray(b1, f32).reshape(MH, 128).T),
        "b2m": np.ascontiguousarray(
            np.asarray(b2, f32).reshape(KC, 128).T),
        "ident": np.eye(128, dtype=f32),
    }
    in_maps = []
    for bidx in range(NCORES):
        xc = np.ascontiguousarray(x[bidx].reshape(C, T))
        m = dict(shared)
        m["x32"] = xc
        m["xb"] = xc.astype(bf16)
        in_maps.append(m)
    return in_maps


def kernel(x, w1, b1, w2, b2, w_down, router_w, w_up):
    x = np.asarray(x)
    assert x.shape == (NCORES, C, 32, 32), x.shape
    nc = _get_nc()
    in_maps = _prep_inputs(x, w1, b1, w2, b2, w_down, router_w, w_up)
    res = run_bass_kernel_spmd(nc, in_maps, list(range(NCORES)))
    outs = np.stack([
        np.asarray(res.results[b]["out"], np.float32).reshape(C, 32, 32)
        for b in range(NCORES)
    ])
    aux = np.stack([np.asarray(res.results[b]["aux"], np.float32)
                    for b in range(NCORES)])  # [8 cores, E, 2]
    tot = aux.sum(axis=0)                     # [E, 2]
    n_tok = np.float32(NCORES * T)
    mean_prob = tot[:, 0] / n_tok
    mean_load = tot[:, 1] / n_tok
    aux_loss = np.float32(E * np.sum(mean_prob * mean_load))
    return outs, aux_loss


# revision 23
# speedup vs baseline: 1.0113x; 1.0113x over previous
"""Trainium2 Bass kernel for nn_AoEBlock (moe_routing).

Sharding: data-parallel over batch B=8 across the 8 NeuronCores (core b
handles image b = 1024 tokens). Weights are replicated per core (bf16 for
the heavy matmuls, f32 where routing precision matters). No collectives;
the aux-loss needs only per-core partial sums ([8,2] floats per core)
which are combined on gather.

Math notes:
 - shared expert + AoE + residual all accumulate into the same PSUM banks:
     out = x + w2t.T@gelu(w1t.T@x + b1) + b2 + wup.T@(gelu(wdt.T@x) * G)
   where G[f, t] = gate(e(f), t) is the dense per-(expert,token) gate
   (0 when the expert is not in the token's top-2), broadcast from an
   [8, T] gate matrix via a tiny selection matmul.
 - router logits are computed exactly in f32 as x.T @ rp where
   rp[e, c] = sum_d router_w[d] * w_down[e*192+d, c] (computed on device
   from the f32 w_down, using a zero-padded per-expert router vector so
   full 128-row k-tiles accumulate cleanly).
 - top-2 renormalized softmax weights: g1 = 1/(1+exp(l2-l1)), g2 = 1-g1
   (exact identity; uses the Exp table already loaded for the softmax).

Schedule: quarter-0 shared-expert work is emitted first so the PE gets
dense matmul work as soon as the first weight chunks land (keeps the HAM
clock warm); router projection + all routing runs next (its DMAs overlap
the shared-expert matmuls); MoE + remaining quarters follow.
"""

import sys
import numpy as np

for _p in ("/opt/trn_rl_repo", "/root/.axon_site"):
    if _p not in sys.path:
        sys.path.insert(0, _p)

import ml_dtypes

import concourse.bass as bass
import concourse.bacc as bacc
import concourse.mybir as mybir
import concourse.tile as tile
from concourse.bass_utils import run_bass_kernel_spmd

F32 = mybir.dt.float32
BF16 = mybir.dt.bfloat16
AF = mybir.ActivationFunctionType
ALU = mybir.AluOpType
AX = mybir.AxisListType

NCORES = 8
C = 768            # model dim
KC = C // 128      # 6 c k-tiles
HID = 3072         # shared expert hidden
MH = HID // 128    # 24 hidden tiles
E = 8              # experts
DL = 192           # d_low
F = E * DL         # 1536 flattened expert-feature dim
JF = F // 128      # 12 feats tiles
T = 1024           # tokens per core
NQ = 2             # token chunks
TQ = T // NQ       # 512 tokens per chunk
BIGNEG = -1.0e30


def build_nc():
    nc = bacc.Bacc("TRN2", target_bir_lowering=False, debug=False,
                   num_devices=NCORES)

    # ---- DRAM parameters (per-core shards) ----
    x32_h = nc.dram_tensor("x32", [C, T], F32, kind="ExternalInput")
    xb_h = nc.dram_tensor("xb", [C, T], BF16, kind="ExternalInput")
    w1t_h = nc.dram_tensor("w1t", [C, HID], BF16, kind="ExternalInput")
    w2t_h = nc.dram_tensor("w2t", [HID, C], BF16, kind="ExternalInput")
    wdt_h = nc.dram_tensor("wdt", [C, F], BF16, kind="ExternalInput")
    wup_h = nc.dram_tensor("wup", [F, C], BF16, kind="ExternalInput")
    wd32_h = nc.dram_tensor("wd32", [F, C], F32, kind="ExternalInput")
    rbig8_h = nc.dram_tensor("rbig8", [128, JF * E], F32, kind="ExternalInput")
    selm_h = nc.dram_tensor("selm", [E, F], BF16, kind="ExternalInput")
    b1m_h = nc.dram_tensor("b1m", [128, MH], F32, kind="ExternalInput")
    b2m_h = nc.dram_tensor("b2m", [128, KC], F32, kind="ExternalInput")
    ident_h = nc.dram_tensor("ident", [128, 128], F32, kind="ExternalInput")
    out_h = nc.dram_tensor("out", [C, T], F32, kind="ExternalOutput")
    aux_h = nc.dram_tensor("aux", [E, 2], F32, kind="ExternalOutput")

    with tile.TileContext(nc) as tc:
        with (
            tc.tile_pool(name="const", bufs=1) as constp,
            tc.tile_pool(name="big", bufs=1) as bigp,
            tc.tile_pool(name="gh", bufs=2) as ghp,
            tc.tile_pool(name="gf", bufs=2) as gfp,
            tc.tile_pool(name="ga", bufs=2) as gap,
            tc.tile_pool(name="ev", bufs=2) as evp,
            tc.tile_pool(name="rt", bufs=2) as rtp,
            tc.tile_pool(name="po", bufs=6, space=bass.MemorySpace.PSUM) as pop,
            tc.tile_pool(name="ph", bufs=2, space=bass.MemorySpace.PSUM) as php,
        ):
            # ---- persistent SBUF tensors ----
            x_sb = constp.tile([128, KC * T], F32, tag="x32")
            xb_sb = constp.tile([128, KC * T], BF16, tag="xb")
            w1t_sb = constp.tile([128, KC * HID], BF16, tag="w1t")
            w2t_sb = constp.tile([128, MH * C], BF16, tag="w2t")
            wdt_sb = constp.tile([128, KC * F], BF16, tag="wdt")
            wup_sb = constp.tile([128, JF * C], BF16, tag="wup")
            # wd32 dies after the router projection; the shared-expert
            # partials are born after it — share one big slot.
            wd32_sb = bigp.tile([128, JF * C], F32, tag="big", name="wd32")
            rbig8_sb = constp.tile([128, JF * E], F32, tag="rbig8")
            selm_sb = constp.tile([E, F], BF16, tag="selm")
            b1_sb = constp.tile([128, MH], F32, tag="b1m")
            b2_sb = constp.tile([128, KC], F32, tag="b2m")
            ident_sb = constp.tile([128, 128], F32, tag="ident")
            rpt_sb = constp.tile([128, KC * E], F32, tag="rpt")
            gate_sb = constp.tile([E, T], BF16, tag="gate")
            ones_sb = constp.tile([128, 1], F32, tag="ones")
            aux_sb = constp.tile([E, 2], F32, tag="auxs")
            shared_sb = bigp.tile([128, KC * T], BF16, tag="big",
                                  name="shared")

            # ---- DMA loads, ordered by first consumption ----
            # Merged 3D-access-pattern transfers keep the per-dma_start issue
            # cost (~0.6us on the sync sequencer) off the critical path.
            xb_dst = xb_sb[:].rearrange("p (k t) -> p k t", k=KC)
            xb_src = xb_h[:].rearrange("(k p) t -> p k t", p=128)
            x32_dst = x_sb[:].rearrange("p (k t) -> p k t", k=KC)
            x32_src = x32_h[:].rearrange("(k p) t -> p k t", p=128)
            w1t_dst = w1t_sb[:].rearrange("p (k w) -> p k w", k=KC)
            w1t_src = w1t_h[:].rearrange("(k p) w -> p k w", p=128)
            w2t_dst = w2t_sb[:].rearrange("p (m c) -> p m c", m=MH)
            w2t_src = w2t_h[:].rearrange("(m p) c -> p m c", p=128)
            wd32_dst = wd32_sb[:].rearrange("p (j c) -> p j c", j=JF)
            wd32_src = wd32_h[:].rearrange("(j p) c -> p j c", p=128)
            wdt_dst = wdt_sb[:].rearrange("p (k f) -> p k f", k=KC)
            wdt_src = wdt_h[:].rearrange("(k p) f -> p k f", p=128)
            wup_dst = wup_sb[:].rearrange("p (j c) -> p j c", j=JF)
            wup_src = wup_h[:].rearrange("(j p) c -> p j c", p=128)

            nc.sync.dma_start(b1_sb[:], b1m_h[:])
            nc.sync.dma_start(xb_dst[:, :, 0:TQ], xb_src[:, :, 0:TQ])
            # w1t / w2t interleaved in hidden-tile (consumption) order
            for g in range(6):
                w0, w1 = g * (HID // 6), (g + 1) * (HID // 6)
                nc.sync.dma_start(w1t_dst[:, :, w0:w1], w1t_src[:, :, w0:w1])
                m0, m1 = g * (MH // 6), (g + 1) * (MH // 6)
                nc.sync.dma_start(w2t_dst[:, m0:m1, :], w2t_src[:, m0:m1, :])
                if g == 0:  # rest of xb right behind the first weight chunk
                    nc.sync.dma_start(xb_dst[:, :, TQ:T], xb_src[:, :, TQ:T])
            nc.vector.memset(ones_sb[:], 1.0)
            nc.vector.memset(aux_sb[:], 0.0)
            nc.sync.dma_start(b2_sb[:], b2m_h[:])
            nc.sync.dma_start(rbig8_sb[:], rbig8_h[:])
            nc.sync.dma_start(selm_sb[:], selm_h[:])
            nc.sync.dma_start(ident_sb[:], ident_h[:])
            # router path + x32 (consumed mid-kernel)
            for g in range(4):
                j0, j1 = g * (JF // 4), (g + 1) * (JF // 4)
                nc.sync.dma_start(wd32_dst[:, j0:j1, :], wd32_src[:, j0:j1, :])
            nc.sync.dma_start(x32_dst[:], x32_src[:])
            # MoE weights (consumed in the back half)
            for g in range(2):
                k0, k1 = g * (KC // 2), (g + 1) * (KC // 2)
                nc.sync.dma_start(wdt_dst[:, k0:k1, :], wdt_src[:, k0:k1, :])
            for g in range(2):
                j0, j1 = g * (JF // 2), (g + 1) * (JF // 2)
                nc.sync.dma_start(wup_dst[:, j0:j1, :], wup_src[:, j0:j1, :])

            # ---- helpers ----
            # Both heavy loops are software-pipelined by one step: the
            # consumer matmuls (mm2 / expert) for step i are emitted after
            # the producer matmuls of step i+1, so the ACT/DVE latency of
            # gelu/gating is covered by PE work and the PE never waits.
            def shared_expert_phase(q, po, out_slice, pending, m_lo=0,
                                    m_hi=MH, flush=False):
                def mm2(m, gh):
                    for ct in range(KC):
                        nc.tensor.matmul(
                            out_slice(ct),
                            w2t_sb[:, m * C + ct * 128: m * C + ct * 128 + 128],
                            gh[:],
                            start=(m == 0),
                            stop=(m == MH - 1),
                        )
                for m in range(m_lo, m_hi):
                    h_ps = php.tile([128, TQ], F32, tag="hf", name=f"h{q}_{m}")
                    for k in range(KC):
                        nc.tensor.matmul(
                            h_ps[:],
                            w1t_sb[:, k * HID + m * 128: k * HID + m * 128 + 128],
                            xb_sb[:, k * T + q * TQ: k * T + q * TQ + TQ],
                            start=(k == 0), stop=(k == KC - 1),
                        )
                    gh = ghp.tile([128, TQ], BF16, tag="gh", name=f"gh{q}_{m}")
                    nc.scalar.activation(gh[:], h_ps[:], AF.Gelu,
                                         bias=b1_sb[:, m:m + 1])
                    if pending:
                        mm2(*pending.pop())
                    pending.append((m, gh))
                if flush:
                    while pending:
                        mm2(*pending.pop())

            def moe_phase(q, po, out_slice):
                pending = []

                def expert(j, ga):
                    for ct in range(KC):
                        nc.tensor.matmul(
                            out_slice(ct),
                            wup_sb[:, j * C + ct * 128: j * C + ct * 128 + 128],
                            ga[:],
                            start=(j == 0),
                            stop=(j == JF - 1),
                        )
                for j in range(JF):
                    f_ps = php.tile([128, TQ], F32, tag="hf", name=f"f{q}_{j}")
                    for k in range(KC):
                        nc.tensor.matmul(
                            f_ps[:],
                            wdt_sb[:, k * F + j * 128: k * F + j * 128 + 128],
                            xb_sb[:, k * T + q * TQ: k * T + q * TQ + TQ],
                            start=(k == 0), stop=(k == KC - 1),
                        )
                    gf = gfp.tile([128, TQ], BF16, tag="gf", name=f"gf{q}_{j}")
                    nc.scalar.activation(gf[:], f_ps[:], AF.Gelu)
                    g_ps = php.tile([128, TQ], F32, tag="hf", name=f"g{q}_{j}")
                    nc.tensor.matmul(g_ps[:],
                                     selm_sb[:, j * 128:(j + 1) * 128],
                                     gate_sb[:, q * TQ:(q + 1) * TQ],
                                     start=True, stop=True)
                    ga = gap.tile([128, TQ], BF16, tag="ga", name=f"ga{q}_{j}")
                    nc.vector.tensor_mul(ga[:], gf[:], g_ps[:])
                    if pending:
                        expert(*pending.pop())
                    pending.append((j, ga))
                while pending:
                    expert(*pending.pop())

            def evict_shared(q, out_slice):
                # park the shared-expert accumulation in SBUF (bf16), freeing
                # the po banks for the next quarter
                for ct in range(KC):
                    nc.scalar.copy(
                        shared_sb[:, ct * T + q * TQ: ct * T + q * TQ + TQ],
                        out_slice(ct))

            def evict_phase(q, po, out_slice):
                # out = moe_psum + x + shared + b2
                for ct in range(KC):
                    ev = evp.tile([128, TQ], F32, tag="ev", name=f"ev{q}_{ct}")
                    nc.vector.scalar_tensor_tensor(
                        ev[:], out_slice(ct), b2_sb[:, ct:ct + 1],
                        x_sb[:, ct * T + q * TQ: ct * T + q * TQ + TQ],
                        ALU.add, ALU.add,
                    )
                    nc.vector.tensor_add(
                        ev[:], ev[:],
                        shared_sb[:, ct * T + q * TQ: ct * T + q * TQ + TQ])
                    nc.sync.dma_start(
                        out_h[ct * 128:(ct + 1) * 128, q * TQ:(q + 1) * TQ],
                        ev[:])

            def make_po(q):
                po = [pop.tile([128, TQ], F32, tag="po", name=f"po{q}_{i}")
                      for i in range(KC)]

                def out_slice(ct):
                    return po[ct][:]
                return po, out_slice

            def router_projection():
                # rp[c, ct*8+e] over one psum accumulation region; rbig8 is
                # zero off-expert, so full 128-row k-tiles accumulate the
                # per-expert d-contraction exactly.
                rp_ps = php.tile([128, KC * E], F32, tag="hf", name="rp_ps")
                for ct in range(KC):
                    for kt in range(JF):
                        nc.tensor.matmul(
                            rp_ps[:, ct * E:(ct + 1) * E],
                            wd32_sb[:, kt * C + ct * 128: kt * C + ct * 128 + 128],
                            rbig8_sb[:, kt * E:(kt + 1) * E],
                            start=(ct == 0 and kt == 0),
                            stop=(ct == KC - 1 and kt == JF - 1),
                        )
                nc.vector.tensor_copy(rpt_sb[:], rp_ps[:])

            Ls = {}

            def logits_stage(tt):
                tok0 = tt * 128
                l_ps = php.tile([128, E], F32, tag="hf", name=f"l{tt}")
                for k in range(KC):
                    nc.tensor.matmul(
                        l_ps[:],
                        x_sb[:, k * T + tok0: k * T + tok0 + 128],
                        rpt_sb[:, k * E:(k + 1) * E],
                        start=(k == 0), stop=(k == KC - 1),
                    )
                L = rtp.tile([128, E], F32, tag="L", name=f"L{tt}", bufs=8)
                nc.vector.tensor_copy(L[:], l_ps[:])
                Ls[tt] = L

            routed = {}

            def route_stage(tt):
                # DVE/ACT only: from logits to token-major gates + aux inputs
                L = Ls[tt]
                m1 = rtp.tile([128, 1], F32, tag="m1", name=f"m1_{tt}")
                nm1 = rtp.tile([128, 1], F32, tag="nm1", name=f"nm1_{tt}")
                nc.vector.tensor_reduce(m1[:], L[:], AX.X, ALU.max)
                nc.vector.tensor_reduce(nm1[:], L[:], AX.X, ALU.max,
                                        negate=True)
                s = rtp.tile([128, E], F32, tag="s", name=f"s{tt}")
                nc.scalar.activation(s[:], L[:], AF.Exp, bias=nm1[:, 0:1])
                Z = rtp.tile([128, 1], F32, tag="Z", name=f"Z{tt}")
                nc.vector.tensor_reduce(Z[:], s[:], AX.X, ALU.add)
                rZ = rtp.tile([128, 1], F32, tag="rZ", name=f"rZ{tt}")
                nc.vector.reciprocal(rZ[:], Z[:])
                probs = rtp.tile([128, E], F32, tag="probs", name=f"pr{tt}", bufs=8)
                nc.vector.tensor_scalar_mul(probs[:], s[:], rZ[:, 0:1])
                mask1 = rtp.tile([128, E], F32, tag="mask1", name=f"mk1{tt}")
                nc.vector.tensor_scalar(mask1[:], L[:], m1[:, 0:1], None,
                                        ALU.is_equal)
                L2 = rtp.tile([128, E], F32, tag="L2", name=f"L2_{tt}")
                nc.vector.scalar_tensor_tensor(L2[:], mask1[:], BIGNEG,
                                               L[:], ALU.mult, ALU.add)
                m2 = rtp.tile([128, 1], F32, tag="m2", name=f"m2_{tt}")
                nc.vector.tensor_reduce(m2[:], L2[:], AX.X, ALU.max)
                mask2 = rtp.tile([128, E], F32, tag="mask2", name=f"mk2{tt}")
                nc.vector.tensor_scalar(mask2[:], L2[:], m2[:, 0:1], None,
                                        ALU.is_equal)
                # g1 = 1/(1+exp(l2-l1)); g2 = 1-g1  (same identity as
                # renormalized top-2 softmax; reuses the Exp table)
                em = rtp.tile([128, 1], F32, tag="em", name=f"em{tt}")
                nc.scalar.activation(em[:], m2[:], AF.Exp, bias=nm1[:, 0:1])
                den = rtp.tile([128, 1], F32, tag="den", name=f"den{tt}")
                nc.vector.tensor_scalar_add(den[:], em[:], 1.0)
                g1 = rtp.tile([128, 1], F32, tag="g1", name=f"g1_{tt}")
                nc.vector.reciprocal(g1[:], den[:])
                g2 = rtp.tile([128, 1], F32, tag="g2", name=f"g2_{tt}")
                nc.vector.tensor_scalar(g2[:], g1[:], -1.0, 1.0,
                                        ALU.mult, ALU.add)
                tmp2 = rtp.tile([128, E], F32, tag="tmp2", name=f"t2_{tt}")
                nc.vector.tensor_scalar_mul(tmp2[:], mask2[:], g2[:, 0:1])
                gtok = rtp.tile([128, E], F32, tag="gtok", name=f"gt{tt}", bufs=8)
                nc.vector.scalar_tensor_tensor(gtok[:], mask1[:],
                                               g1[:, 0:1], tmp2[:],
                                               ALU.mult, ALU.add)
                oh = rtp.tile([128, E], F32, tag="oh", name=f"oh{tt}", bufs=8)
                nc.vector.tensor_add(oh[:], mask1[:], mask2[:])
                routed[tt] = (gtok, probs, oh)

            def finish_stage(tt):
                # PE transpose of gates into gate_sb + aux partial sums
                gtok, probs, oh = routed[tt]
                tr_ps = php.tile([E, 128], F32, tag="hf", name=f"tr{tt}")
                nc.tensor.transpose(tr_ps[:], gtok[:], ident_sb[:])
                nc.vector.tensor_copy(gate_sb[:, tt * 128:(tt + 1) * 128],
                                      tr_ps[:])
                aux_ps = php.tile([E, 2], F32, tag="hf", name=f"ax{tt}")
                nc.tensor.matmul(aux_ps[:, 0:1], probs[:], ones_sb[:],
                                 start=True, stop=True)
                nc.tensor.matmul(aux_ps[:, 1:2], oh[:], ones_sb[:],
                                 start=True, stop=True)
                nc.vector.tensor_add(aux_sb[:], aux_sb[:], aux_ps[:])

            # ---- emission schedule ----
            # Quarter-0 shared expert gives the PE dense work immediately;
            # router-projection / logits / routing slot between its segments
            # so their DMA + DVE latencies hide under shared-expert matmuls.
            # Routing chains for token tiles 2-7 (used by quarters 1-3) are
            # emitted after quarter-1's shared-expert loop starts so the DVE
            # never delays quarter-0's gate multiplies.
            # Phase 1: shared expert for ALL quarters back-to-back — pure
            # dense PE work paced only by the w1t/w2t DMA stream. Each
            # quarter's accumulation parks in shared_sb, freeing the po
            # banks. The router (projection, logits, chains, gate builds)
            # slots into the stream where its inputs have surely arrived,
            # long before the MoE phase needs the gates.
            for q in range(NQ):
                po, osl = make_po(q)
                pend = []
                if q == 0:
                    shared_expert_phase(q, po, osl, pend, 0, 12)
                    router_projection()
                    shared_expert_phase(q, po, osl, pend, 12, 24, flush=True)
                else:
                    shared_expert_phase(q, po, osl, pend, 0, 4)
                    for tt in range(4):
                        logits_stage(tt)
                    shared_expert_phase(q, po, osl, pend, 4, 8)
                    for tt in range(4, T // 128):
                        logits_stage(tt)
                    shared_expert_phase(q, po, osl, pend, 8, 12)
                    for tt in range(T // 128):
                        route_stage(tt)
                    shared_expert_phase(q, po, osl, pend, 12, 24, flush=True)
                evict_shared(q, osl)
                if q == NQ - 1:
                    for tt in range(T // 128):
                        finish_stage(tt)

            # Phase 2: MoE for all quarters (weights + gates all resident)
            for q in range(NQ):
                po, osl = make_po(q)
                moe_phase(q, po, osl)
                evict_phase(q, po, osl)

            nc.sync.dma_start(aux_h[:], aux_sb[:])

    nc.compile()
    return nc


_CACHE = {}


def _get_nc():
    if "nc" not in _CACHE:
        _CACHE["nc"] = build_nc()
    return _CACHE["nc"]


def _prep_inputs(x, w1, b1, w2, b2, w_down, router_w, w_up):
    bf16 = ml_dtypes.bfloat16
    f32 = np.float32
    x = np.asarray(x, f32)
    # rbig8[p, kt*8+e] = router_w[(kt*128+p) % 192] if (kt*128+p)//192 == e
    flat = np.arange(128)[:, None] + 128 * np.arange(JF)[None, :]  # [128, JF]
    rvals = np.asarray(router_w, f32)[0][flat % DL]                # [128, JF]
    emask = (flat // DL)[:, :, None] == np.arange(E)[None, None, :]
    rbig8 = (rvals[:, :, None] * emask).reshape(128, JF * E).astype(f32)
    shared = {
        "w1t": np.ascontiguousarray(np.asarray(w1, f32).T.astype(bf16)),
        "w2t": np.ascontiguousarray(np.asarray(w2, f32).T.astype(bf16)),
        "wdt": np.ascontiguousarray(np.asarray(w_down, f32).T.astype(bf16)),
        "wup": np.ascontiguousarray(
            np.asarray(w_up, f32).reshape(F, C).astype(bf16)),
        "wd32": np.ascontiguousarray(np.asarray(w_down, f32)),
        "rbig8": np.ascontiguousarray(rbig8),
        "selm": np.ascontiguousarray(
            (np.arange(F)[None, :] // DL == np.arange(E)[:, None])
            .astype(bf16)),
        "b1m": np.ascontiguousarray(
            np.asarray(b1, f32).reshape(MH, 128).T),
        "b2m": np.ascontiguousarray(
            np.asarray(b2, f32).reshape(KC, 128).T),
        "ident": np.eye(128, dtype=f32),
    }
    in_maps = []
    for bidx in range(NCORES):
        xc = np.ascontiguousarray(x[bidx].reshape(C, T))
        m = dict(shared)
        m["x32"] = xc
        m["xb"] = xc.astype(bf16)
        in_maps.append(m)
    return in_maps


def kernel(x, w1, b1, w2, b2, w_down, router_w, w_up):
    x = np.asarray(x)
    assert x.shape == (NCORES, C, 32, 32), x.shape
    nc = _get_nc()
    in_maps = _prep_inputs(x, w1, b1, w2, b2, w_down, router_w, w_up)
    res = run_bass_kernel_spmd(nc, in_maps, list(range(NCORES)))
    outs = np.stack([
        np.asarray(res.results[b]["out"], np.float32).reshape(C, 32, 32)
        for b in range(NCORES)
    ])
    aux = np.stack([np.asarray(res.results[b]["aux"], np.float32)
                    for b in range(NCORES)])  # [8 cores, E, 2]
    tot = aux.sum(axis=0)                     # [E, 2]
    n_tok = np.float32(NCORES * T)
    mean_prob = tot[:, 0] / n_tok
    mean_load = tot[:, 1] / n_tok
    aux_loss = np.float32(E * np.sum(mean_prob * mean_load))
    return outs, aux_loss


# revision 25
# speedup vs baseline: 1.1990x; 1.1856x over previous
"""Trainium2 Bass kernel for nn_AoEBlock (moe_routing).

Sharding: data-parallel over batch B=8 across the 8 NeuronCores (core b
handles image b = 1024 tokens). Weights are replicated per core (bf16 for
the heavy matmuls, f32 where routing precision matters). No collectives;
the aux-loss needs only per-core partial sums ([8,2] floats per core)
which are combined on gather.

Math notes:
 - shared expert + AoE + residual all accumulate into the same PSUM banks:
     out = x + w2t.T@gelu(w1t.T@x + b1) + b2 + wup.T@(gelu(wdt.T@x) * G)
   where G[f, t] = gate(e(f), t) is the dense per-(expert,token) gate
   (0 when the expert is not in the token's top-2), broadcast from an
   [8, T] gate matrix via a tiny selection matmul.
 - router logits are computed exactly in f32 as x.T @ rp where
   rp[e, c] = sum_d router_w[d] * w_down[e*192+d, c] (computed on device
   from the f32 w_down, using a zero-padded per-expert router vector so
   full 128-row k-tiles accumulate cleanly).
 - top-2 renormalized softmax weights: g1 = 1/(1+exp(l2-l1)), g2 = 1-g1
   (exact identity; uses the Exp table already loaded for the softmax).

Schedule: quarter-0 shared-expert work is emitted first so the PE gets
dense matmul work as soon as the first weight chunks land (keeps the HAM
clock warm); router projection + all routing runs next (its DMAs overlap
the shared-expert matmuls); MoE + remaining quarters follow.
"""

import sys
import numpy as np

for _p in ("/opt/trn_rl_repo", "/root/.axon_site"):
    if _p not in sys.path:
        sys.path.insert(0, _p)

import ml_dtypes

import concourse.bass as bass
import concourse.bacc as bacc
import concourse.mybir as mybir
import concourse.tile as tile
from concourse.bass_utils import run_bass_kernel_spmd

F32 = mybir.dt.float32
BF16 = mybir.dt.bfloat16
AF = mybir.ActivationFunctionType
ALU = mybir.AluOpType
AX = mybir.AxisListType

NCORES = 8
C = 768            # model dim
KC = C // 128      # 6 c k-tiles
HID = 3072         # shared expert hidden
MH = HID // 128    # 24 hidden tiles
E = 8              # experts
DL = 192           # d_low
F = E * DL         # 1536 flattened expert-feature dim
JF = F // 128      # 12 feats tiles
T = 1024           # tokens per core
NQ = 2             # token chunks
TQ = T // NQ       # 512 tokens per chunk
BIGNEG = -1.0e30


def build_nc():
    nc = bacc.Bacc("TRN2", target_bir_lowering=False, debug=False,
                   num_devices=NCORES)

    # ---- DRAM parameters (per-core shards) ----
    x32_h = nc.dram_tensor("x32", [C, T], F32, kind="ExternalInput")
    xb_h = nc.dram_tensor("xb", [C, T], BF16, kind="ExternalInput")
    w1t_h = nc.dram_tensor("w1t", [C, HID], BF16, kind="ExternalInput")
    w2t_h = nc.dram_tensor("w2t", [HID, C], BF16, kind="ExternalInput")
    wdt_h = nc.dram_tensor("wdt", [C, F], BF16, kind="ExternalInput")
    wup_h = nc.dram_tensor("wup", [F, C], BF16, kind="ExternalInput")
    wd32_h = nc.dram_tensor("wd32", [F, C], F32, kind="ExternalInput")
    rbig8_h = nc.dram_tensor("rbig8", [128, JF * E], F32, kind="ExternalInput")
    selm_h = nc.dram_tensor("selm", [E, F], BF16, kind="ExternalInput")
    b1m_h = nc.dram_tensor("b1m", [128, MH], F32, kind="ExternalInput")
    b2m_h = nc.dram_tensor("b2m", [128, KC], F32, kind="ExternalInput")
    ident_h = nc.dram_tensor("ident", [128, 128], F32, kind="ExternalInput")
    out_h = nc.dram_tensor("out", [C, T], F32, kind="ExternalOutput")
    aux_h = nc.dram_tensor("aux", [E, 2], F32, kind="ExternalOutput")

    with tile.TileContext(nc) as tc:
        with (
            tc.tile_pool(name="const", bufs=1) as constp,
            tc.tile_pool(name="big", bufs=1) as bigp,
            tc.tile_pool(name="gh", bufs=3) as ghp,
            tc.tile_pool(name="gf", bufs=2) as gfp,
            tc.tile_pool(name="ga", bufs=3) as gap,
            tc.tile_pool(name="ev", bufs=2) as evp,
            tc.tile_pool(name="rt", bufs=2) as rtp,
            tc.tile_pool(name="po", bufs=6, space=bass.MemorySpace.PSUM) as pop,
            tc.tile_pool(name="ph", bufs=2, space=bass.MemorySpace.PSUM) as php,
        ):
            # ---- persistent SBUF tensors ----
            x_sb = constp.tile([128, KC * T], F32, tag="x32")
            xb_sb = constp.tile([128, KC * T], BF16, tag="xb")
            w1t_sb = constp.tile([128, KC * HID], BF16, tag="w1t")
            w2t_sb = constp.tile([128, MH * C], BF16, tag="w2t")
            wdt_sb = constp.tile([128, KC * F], BF16, tag="wdt")
            wup_sb = constp.tile([128, JF * C], BF16, tag="wup")
            # wd32 dies after the router projection; the shared-expert
            # partials are born after it — share one big slot.
            wd32_sb = bigp.tile([128, JF * C], F32, tag="big", name="wd32")
            rbig8_sb = constp.tile([128, JF * E], F32, tag="rbig8")
            selm_sb = constp.tile([E, F], BF16, tag="selm")
            b1_sb = constp.tile([128, MH], F32, tag="b1m")
            b2_sb = constp.tile([128, KC], F32, tag="b2m")
            ident_sb = constp.tile([128, 128], F32, tag="ident")
            rpt_sb = constp.tile([128, KC * E], F32, tag="rpt")
            gate_sb = constp.tile([E, T], BF16, tag="gate")
            ones_sb = constp.tile([128, 1], F32, tag="ones")
            aux_sb = constp.tile([E, 2], F32, tag="auxs")
            shared_sb = bigp.tile([128, KC * T], BF16, tag="big",
                                  name="shared")

            # ---- DMA loads, ordered by first consumption ----
            # Merged 3D-access-pattern transfers keep the per-dma_start issue
            # cost (~0.6us on the sync sequencer) off the critical path.
            xb_dst = xb_sb[:].rearrange("p (k t) -> p k t", k=KC)
            xb_src = xb_h[:].rearrange("(k p) t -> p k t", p=128)
            x32_dst = x_sb[:].rearrange("p (k t) -> p k t", k=KC)
            x32_src = x32_h[:].rearrange("(k p) t -> p k t", p=128)
            w1t_dst = w1t_sb[:].rearrange("p (k w) -> p k w", k=KC)
            w1t_src = w1t_h[:].rearrange("(k p) w -> p k w", p=128)
            w2t_dst = w2t_sb[:].rearrange("p (m c) -> p m c", m=MH)
            w2t_src = w2t_h[:].rearrange("(m p) c -> p m c", p=128)
            wd32_dst = wd32_sb[:].rearrange("p (j c) -> p j c", j=JF)
            wd32_src = wd32_h[:].rearrange("(j p) c -> p j c", p=128)
            wdt_dst = wdt_sb[:].rearrange("p (k f) -> p k f", k=KC)
            wdt_src = wdt_h[:].rearrange("(k p) f -> p k f", p=128)
            wup_dst = wup_sb[:].rearrange("p (j c) -> p j c", j=JF)
            wup_src = wup_h[:].rearrange("(j p) c -> p j c", p=128)

            nc.sync.dma_start(b1_sb[:], b1m_h[:])
            nc.sync.dma_start(xb_dst[:, :, 0:TQ], xb_src[:, :, 0:TQ])
            # w1t / w2t interleaved in hidden-tile (consumption) order
            for g in range(6):
                w0, w1 = g * (HID // 6), (g + 1) * (HID // 6)
                nc.sync.dma_start(w1t_dst[:, :, w0:w1], w1t_src[:, :, w0:w1])
                m0, m1 = g * (MH // 6), (g + 1) * (MH // 6)
                nc.sync.dma_start(w2t_dst[:, m0:m1, :], w2t_src[:, m0:m1, :])
                if g == 0:  # rest of xb right behind the first weight chunk
                    nc.sync.dma_start(xb_dst[:, :, TQ:T], xb_src[:, :, TQ:T])
            nc.vector.memset(ones_sb[:], 1.0)
            nc.vector.memset(aux_sb[:], 0.0)
            nc.sync.dma_start(b2_sb[:], b2m_h[:])
            nc.sync.dma_start(rbig8_sb[:], rbig8_h[:])
            nc.sync.dma_start(selm_sb[:], selm_h[:])
            nc.sync.dma_start(ident_sb[:], ident_h[:])
            # router path + x32 (consumed mid-kernel)
            for g in range(4):
                j0, j1 = g * (JF // 4), (g + 1) * (JF // 4)
                nc.sync.dma_start(wd32_dst[:, j0:j1, :], wd32_src[:, j0:j1, :])
            nc.sync.dma_start(x32_dst[:], x32_src[:])
            # MoE weights (consumed in the back half)
            for g in range(2):
                k0, k1 = g * (KC // 2), (g + 1) * (KC // 2)
                nc.sync.dma_start(wdt_dst[:, k0:k1, :], wdt_src[:, k0:k1, :])
            for g in range(2):
                j0, j1 = g * (JF // 2), (g + 1) * (JF // 2)
                nc.sync.dma_start(wup_dst[:, j0:j1, :], wup_src[:, j0:j1, :])

            # ---- helpers ----
            # Both heavy loops are software-pipelined by one step: the
            # consumer matmuls (mm2 / expert) for step i are emitted after
            # the producer matmuls of step i+1, so the ACT/DVE latency of
            # gelu/gating is covered by PE work and the PE never waits.
            def shared_expert_phase(q, po, out_slice, pending, m_lo=0,
                                    m_hi=MH, flush=False):
                def mm2(m, gh):
                    for ct in range(KC):
                        nc.tensor.matmul(
                            out_slice(ct),
                            w2t_sb[:, m * C + ct * 128: m * C + ct * 128 + 128],
                            gh[:],
                            start=(m == 0),
                            stop=(m == MH - 1),
                        )
                for m in range(m_lo, m_hi):
                    h_ps = php.tile([128, TQ], F32, tag="hf", name=f"h{q}_{m}")
                    for k in range(KC):
                        nc.tensor.matmul(
                            h_ps[:],
                            w1t_sb[:, k * HID + m * 128: k * HID + m * 128 + 128],
                            xb_sb[:, k * T + q * TQ: k * T + q * TQ + TQ],
                            start=(k == 0), stop=(k == KC - 1),
                        )
                    gh = ghp.tile([128, TQ], BF16, tag="gh", name=f"gh{q}_{m}")
                    nc.scalar.activation(gh[:], h_ps[:], AF.Gelu,
                                         bias=b1_sb[:, m:m + 1])
                    if len(pending) >= 2:
                        mm2(*pending.pop(0))
                    pending.append((m, gh))
                if flush:
                    while pending:
                        mm2(*pending.pop(0))

            def moe_phase(q, po, out_slice):
                pending = []

                def expert(j, ga):
                    for ct in range(KC):
                        nc.tensor.matmul(
                            out_slice(ct),
                            wup_sb[:, j * C + ct * 128: j * C + ct * 128 + 128],
                            ga[:],
                            start=(j == 0),
                            stop=(j == JF - 1),
                        )
                for j in range(JF):
                    f_ps = php.tile([128, TQ], F32, tag="hf", name=f"f{q}_{j}")
                    for k in range(KC):
                        nc.tensor.matmul(
                            f_ps[:],
                            wdt_sb[:, k * F + j * 128: k * F + j * 128 + 128],
                            xb_sb[:, k * T + q * TQ: k * T + q * TQ + TQ],
                            start=(k == 0), stop=(k == KC - 1),
                        )
                    gf = gfp.tile([128, TQ], BF16, tag="gf", name=f"gf{q}_{j}")
                    nc.scalar.activation(gf[:], f_ps[:], AF.Gelu)
                    g_ps = php.tile([128, TQ], F32, tag="hf", name=f"g{q}_{j}")
                    nc.tensor.matmul(g_ps[:],
                                     selm_sb[:, j * 128:(j + 1) * 128],
                                     gate_sb[:, q * TQ:(q + 1) * TQ],
                                     start=True, stop=True)
                    ga = gap.tile([128, TQ], BF16, tag="ga", name=f"ga{q}_{j}")
                    nc.vector.tensor_mul(ga[:], gf[:], g_ps[:])
                    if len(pending) >= 2:
                        expert(*pending.pop(0))
                    pending.append((j, ga))
                while pending:
                    expert(*pending.pop(0))

            def evict_shared(q, out_slice):
                # park the shared-expert accumulation in SBUF (bf16), freeing
                # the po banks for the next quarter
                for ct in range(KC):
                    nc.scalar.copy(
                        shared_sb[:, ct * T + q * TQ: ct * T + q * TQ + TQ],
                        out_slice(ct))

            def fold_shared_into_x(q):
                # x_sb <- x + shared (in place, after logits no longer needs
                # raw x); makes the final eviction a single DVE op
                for ct in range(KC):
                    sl = slice(ct * T + q * TQ, ct * T + q * TQ + TQ)
                    nc.vector.tensor_add(x_sb[:, sl], x_sb[:, sl],
                                         shared_sb[:, sl])

            def evict_phase(q, po, out_slice):
                # out = moe_psum + b2 + (x + shared)
                for ct in range(KC):
                    ev = evp.tile([128, TQ], F32, tag="ev", name=f"ev{q}_{ct}")
                    nc.vector.scalar_tensor_tensor(
                        ev[:], out_slice(ct), b2_sb[:, ct:ct + 1],
                        x_sb[:, ct * T + q * TQ: ct * T + q * TQ + TQ],
                        ALU.add, ALU.add,
                    )
                    nc.sync.dma_start(
                        out_h[ct * 128:(ct + 1) * 128, q * TQ:(q + 1) * TQ],
                        ev[:])

            def make_po(q):
                po = [pop.tile([128, TQ], F32, tag="po", name=f"po{q}_{i}")
                      for i in range(KC)]

                def out_slice(ct):
                    return po[ct][:]
                return po, out_slice

            def router_projection():
                # rp[c, ct*8+e] over one psum accumulation region; rbig8 is
                # zero off-expert, so full 128-row k-tiles accumulate the
                # per-expert d-contraction exactly.
                rp_ps = php.tile([128, KC * E], F32, tag="hf", name="rp_ps")
                for ct in range(KC):
                    for kt in range(JF):
                        nc.tensor.matmul(
                            rp_ps[:, ct * E:(ct + 1) * E],
                            wd32_sb[:, kt * C + ct * 128: kt * C + ct * 128 + 128],
                            rbig8_sb[:, kt * E:(kt + 1) * E],
                            start=(ct == 0 and kt == 0),
                            stop=(ct == KC - 1 and kt == JF - 1),
                        )
                nc.vector.tensor_copy(rpt_sb[:], rp_ps[:])

            Ls = {}

            def logits_stage(tt):
                tok0 = tt * 128
                l_ps = php.tile([128, E], F32, tag="hf", name=f"l{tt}")
                for k in range(KC):
                    nc.tensor.matmul(
                        l_ps[:],
                        x_sb[:, k * T + tok0: k * T + tok0 + 128],
                        rpt_sb[:, k * E:(k + 1) * E],
                        start=(k == 0), stop=(k == KC - 1),
                    )
                L = rtp.tile([128, E], F32, tag="L", name=f"L{tt}", bufs=8)
                nc.vector.tensor_copy(L[:], l_ps[:])
                Ls[tt] = L

            routed = {}

            def route_stage(tt):
                # DVE/ACT only: from logits to token-major gates + aux inputs
                L = Ls[tt]
                m1 = rtp.tile([128, 1], F32, tag="m1", name=f"m1_{tt}")
                nm1 = rtp.tile([128, 1], F32, tag="nm1", name=f"nm1_{tt}")
                nc.vector.tensor_reduce(m1[:], L[:], AX.X, ALU.max)
                nc.vector.tensor_reduce(nm1[:], L[:], AX.X, ALU.max,
                                        negate=True)
                s = rtp.tile([128, E], F32, tag="s", name=f"s{tt}")
                nc.scalar.activation(s[:], L[:], AF.Exp, bias=nm1[:, 0:1])
                Z = rtp.tile([128, 1], F32, tag="Z", name=f"Z{tt}")
                nc.vector.tensor_reduce(Z[:], s[:], AX.X, ALU.add)
                rZ = rtp.tile([128, 1], F32, tag="rZ", name=f"rZ{tt}")
                nc.vector.reciprocal(rZ[:], Z[:])
                probs = rtp.tile([128, E], F32, tag="probs", name=f"pr{tt}", bufs=8)
                nc.vector.tensor_scalar_mul(probs[:], s[:], rZ[:, 0:1])
                mask1 = rtp.tile([128, E], F32, tag="mask1", name=f"mk1{tt}")
                nc.vector.tensor_scalar(mask1[:], L[:], m1[:, 0:1], None,
                                        ALU.is_equal)
                L2 = rtp.tile([128, E], F32, tag="L2", name=f"L2_{tt}")
                nc.vector.scalar_tensor_tensor(L2[:], mask1[:], BIGNEG,
                                               L[:], ALU.mult, ALU.add)
                m2 = rtp.tile([128, 1], F32, tag="m2", name=f"m2_{tt}")
                nc.vector.tensor_reduce(m2[:], L2[:], AX.X, ALU.max)
                mask2 = rtp.tile([128, E], F32, tag="mask2", name=f"mk2{tt}")
                nc.vector.tensor_scalar(mask2[:], L2[:], m2[:, 0:1], None,
                                        ALU.is_equal)
                # g1 = 1/(1+exp(l2-l1)); g2 = 1-g1  (same identity as
                # renormalized top-2 softmax; reuses the Exp table)
                em = rtp.tile([128, 1], F32, tag="em", name=f"em{tt}")
                nc.scalar.activation(em[:], m2[:], AF.Exp, bias=nm1[:, 0:1])
                den = rtp.tile([128, 1], F32, tag="den", name=f"den{tt}")
                nc.vector.tensor_scalar_add(den[:], em[:], 1.0)
                g1 = rtp.tile([128, 1], F32, tag="g1", name=f"g1_{tt}")
                nc.vector.reciprocal(g1[:], den[:])
                g2 = rtp.tile([128, 1], F32, tag="g2", name=f"g2_{tt}")
                nc.vector.tensor_scalar(g2[:], g1[:], -1.0, 1.0,
                                        ALU.mult, ALU.add)
                tmp2 = rtp.tile([128, E], F32, tag="tmp2", name=f"t2_{tt}")
                nc.vector.tensor_scalar_mul(tmp2[:], mask2[:], g2[:, 0:1])
                gtok = rtp.tile([128, E], F32, tag="gtok", name=f"gt{tt}", bufs=8)
                nc.vector.scalar_tensor_tensor(gtok[:], mask1[:],
                                               g1[:, 0:1], tmp2[:],
                                               ALU.mult, ALU.add)
                oh = rtp.tile([128, E], F32, tag="oh", name=f"oh{tt}", bufs=8)
                nc.vector.tensor_add(oh[:], mask1[:], mask2[:])
                routed[tt] = (gtok, probs, oh)

            def finish_stage(tt):
                # PE transpose of gates into gate_sb + aux partial sums
                gtok, probs, oh = routed[tt]
                tr_ps = php.tile([E, 128], F32, tag="hf", name=f"tr{tt}")
                nc.tensor.transpose(tr_ps[:], gtok[:], ident_sb[:])
                nc.vector.tensor_copy(gate_sb[:, tt * 128:(tt + 1) * 128],
                                      tr_ps[:])
                aux_ps = php.tile([E, 2], F32, tag="hf", name=f"ax{tt}")
                nc.tensor.matmul(aux_ps[:, 0:1], probs[:], ones_sb[:],
                                 start=True, stop=True)
                nc.tensor.matmul(aux_ps[:, 1:2], oh[:], ones_sb[:],
                                 start=True, stop=True)
                nc.vector.tensor_add(aux_sb[:], aux_sb[:], aux_ps[:])

            # ---- emission schedule ----
            # Quarter-0 shared expert gives the PE dense work immediately;
            # router-projection / logits / routing slot between its segments
            # so their DMA + DVE latencies hide under shared-expert matmuls.
            # Routing chains for token tiles 2-7 (used by quarters 1-3) are
            # emitted after quarter-1's shared-expert loop starts so the DVE
            # never delays quarter-0's gate multiplies.
            # Phase 1: shared expert for ALL quarters back-to-back — pure
            # dense PE work paced only by the w1t/w2t DMA stream. Each
            # quarter's accumulation parks in shared_sb, freeing the po
            # banks. The router (projection, logits, chains, gate builds)
            # slots into the stream where its inputs have surely arrived,
            # long before the MoE phase needs the gates.
            for q in range(NQ):
                po, osl = make_po(q)
                pend = []
                if q == 0:
                    shared_expert_phase(q, po, osl, pend, 0, 12)
                    router_projection()
                    shared_expert_phase(q, po, osl, pend, 12, 24, flush=True)
                else:
                    shared_expert_phase(q, po, osl, pend, 0, 4)
                    for tt in range(4):
                        logits_stage(tt)
                    shared_expert_phase(q, po, osl, pend, 4, 8)
                    for tt in range(4, T // 128):
                        logits_stage(tt)
                    shared_expert_phase(q, po, osl, pend, 8, 12)
                    for tt in range(T // 128):
                        route_stage(tt)
                    shared_expert_phase(q, po, osl, pend, 12, 24, flush=True)
                evict_shared(q, osl)
                if q == NQ - 1:
                    for tt in range(T // 128):
                        finish_stage(tt)

            # Phase 2: MoE for all quarters (weights + gates all resident)
            for q in range(NQ):
                fold_shared_into_x(q)
                po, osl = make_po(q)
                moe_phase(q, po, osl)
                evict_phase(q, po, osl)

            nc.sync.dma_start(aux_h[:], aux_sb[:])

    nc.compile()
    return nc


_CACHE = {}


def _get_nc():
    if "nc" not in _CACHE:
        _CACHE["nc"] = build_nc()
    return _CACHE["nc"]


def _prep_inputs(x, w1, b1, w2, b2, w_down, router_w, w_up):
    bf16 = ml_dtypes.bfloat16
    f32 = np.float32
    x = np.asarray(x, f32)
    # rbig8[p, kt*8+e] = router_w[(kt*128+p) % 192] if (kt*128+p)//192 == e
    flat = np.arange(128)[:, None] + 128 * np.arange(JF)[None, :]  # [128, JF]
    rvals = np.asarray(router_w, f32)[0][flat % DL]                # [128, JF]
    emask = (flat // DL)[:, :, None] == np.arange(E)[None, None, :]
    rbig8 = (rvals[:, :, None] * emask).reshape(128, JF * E).astype(f32)
    shared = {
        "w1t": np.ascontiguousarray(np.asarray(w1, f32).T.astype(bf16)),
        "w2t": np.ascontiguousarray(np.asarray(w2, f32).T.astype(bf16)),
        "wdt": np.ascontiguousarray(np.asarray(w_down, f32).T.astype(bf16)),
        "wup": np.ascontiguousarray(
            np.asarray(w_up, f32).reshape(F, C).astype(bf16)),
        "wd32": np.ascontiguousarray(np.asarray(w_down, f32)),
        "rbig8": np.ascontiguousarray(rbig8),
        "selm": np.ascontiguousarray(
            (np.arange(F)[None, :] // DL == np.arange(E)[:, None])
            .astype(bf16)),
        "b1m": np.ascontiguousarray(
            np.asarray(b1, f32).reshape(MH, 128).T),
        "b2m": np.ascontiguousarray(
            np.asarray(b2, f32).reshape(KC, 128).T),
        "ident": np.eye(128, dtype=f32),
    }
    in_maps = []
    for bidx in range(NCORES):
        xc = np.ascontiguousarray(x[bidx].reshape(C, T))
        m = dict(shared)
        m["x32"] = xc
        m["xb"] = xc.astype(bf16)
        in_maps.append(m)
    return in_maps


def kernel(x, w1, b1, w2, b2, w_down, router_w, w_up):
    x = np.asarray(x)
    assert x.shape == (NCORES, C, 32, 32), x.shape
    nc = _get_nc()
    in_maps = _prep_inputs(x, w1, b1, w2, b2, w_down, router_w, w_up)
    res = run_bass_kernel_spmd(nc, in_maps, list(range(NCORES)))
    outs = np.stack([
        np.asarray(res.results[b]["out"], np.float32).reshape(C, 32, 32)
        for b in range(NCORES)
    ])
    aux = np.stack([np.asarray(res.results[b]["aux"], np.float32)
                    for b in range(NCORES)])  # [8 cores, E, 2]
    tot = aux.sum(axis=0)                     # [E, 2]
    n_tok = np.float32(NCORES * T)
    mean_prob = tot[:, 0] / n_tok
    mean_load = tot[:, 1] / n_tok
    aux_loss = np.float32(E * np.sum(mean_prob * mean_load))
    return outs, aux_loss
